# revision 1
# baseline (speedup 1.0000x reference)
"""Grouped-scale dequant GEMM (AxCoreLinearFP16) on 8 Trainium2 NeuronCores.

y[b,s,o] = sum_i x[b,s,i] * (weight[o,i] * scales[o, i//128])

Strategy: data-parallel over the flattened (b*s) rows — each core gets a
[1024, 4096] x-shard and the full weight/scales (no collectives). Per core:
  - x^T resident in SBUF via two 4 MiB DMA transposes (contraction dim on
    partitions: xT[p, ko, m] = x[m, ko*128+p])
  - w^T o-panels (512 wide) DMA-transposed in two 2 MiB chunks each; the
    128-wide k-chunk is exactly one quant group, so dequant needs one
    scales row per (o-chunk, k-chunk) broadcast across partitions. The
    broadcast runs as a rank-1 PE matmul (ones^T x row -> PSUM) and one
    in-place DVE multiply — broadcast-shaped DMAs (step-0 partition APs)
    and extra DRAM round-trips measurably stall the DMA pipeline on HW.
  - scales^T itself is built once on-chip (PE transposes of the [4096, 32]
    scales) and bounced through DRAM so each o-chunk's rows can be
    reloaded contiguously onto partition 0 for the rank-1 matmuls.
  - PE matmul accumulates over the 32 k-chunks into PSUM [128, 512] fp32;
    PSUM is evicted with a casting ACT copy and DMA'd out.

Workarounds for this environment's toolchain:
  - walrus here accepts only ONE sync-wait per instruction: extra waits are
    peeled onto same-engine NoOps (_split_multiwait_insts)
  - InstPartitionBroadcast ("ISA wrong length") and --enable-ldw-opt=true
    do not codegen; both are avoided.

Self-contained: hardcodes shapes from the problem spec.
"""

import sys

for _p in ("/opt/trn_rl_repo",):
    if _p not in sys.path:
        sys.path.insert(0, _p)

from contextlib import ExitStack

import numpy as np

import concourse.bass as bass
import concourse.mybir as mybir
import concourse.tile as tile
import bass_rust
from concourse.masks import make_identity


FP16 = mybir.dt.float16
FP32 = mybir.dt.float32

P = 128
NCORES = 8
B, S, IN, OUT = 4, 2048, 4096, 4096
GROUP = 128
M = B * S // NCORES          # 1024 rows of x per core
KO = IN // P                 # 32 k-chunks == quant groups
OC = 512                     # o-chunk (matmul free dim)
NOC = OUT // OC              # 8
MT = M // P                  # 8 m-tiles

_RUNNER = None


def _split_multiwait_insts(nc):
    """This env's walrus CoreV3 codegen accepts only one sync-wait per
    instruction; Tile's tail drain can carry one per DMAHW sem lane.
    Peel extra waits onto same-engine NoOps inserted just before."""
    ctr = 0
    for f in nc.m.functions:
        for bb in f.blocks:
            new = []
            for inst in bb.instructions:
                si = inst.sync_info
                if si is not None and si.on_wait and len(si.on_wait) > 1:
                    waits = list(si.on_wait)
                    for w in waits[:-1]:
                        ctr += 1
                        new.append(bass_rust.InstNoOp(
                            name=f"I-waitsplit-{ctr}",
                            engine=inst.engine,
                            sync_info=bass_rust.SyncInfo(on_wait=[w], on_update=[]),
                        ))
                    inst.sync_info = bass_rust.SyncInfo(
                        on_wait=[waits[-1]], on_update=list(si.on_update or [])
                    )
                new.append(inst)
            bb.instructions = new
    return ctr


def _build(M=M, IN=IN, OUT=OUT, wdeq_bufs=None, split_waits=True):
    KO = IN // P
    NOC = OUT // OC
    MT = M // P
    nc = bass.Bass()
    x = nc.declare_dram_parameter("x", [M, IN], FP16, isOutput=False)
    w = nc.declare_dram_parameter("w", [OUT, IN], FP16, isOutput=False)
    s = nc.declare_dram_parameter("s", [OUT, KO], FP16, isOutput=False)
    y = nc.declare_dram_parameter("y", [M, OUT], FP16, isOutput=True)

    with tile.TileContext(nc) as tc, ExitStack() as ctx:
        const = ctx.enter_context(tc.tile_pool(name="const", bufs=1))
        scps = ctx.enter_context(tc.tile_pool(name="scps", bufs=2, space="PSUM"))
        dramp = ctx.enter_context(tc.tile_pool(name="dramp", bufs=1, space="DRAM"))
        xTp = ctx.enter_context(tc.tile_pool(name="xTp", bufs=1))
        wraw = ctx.enter_context(tc.tile_pool(name="wraw", bufs=2))
        scp0 = ctx.enter_context(tc.tile_pool(name="scp0", bufs=2))
        psb_pool = ctx.enter_context(tc.tile_pool(name="psb", bufs=2, space="PSUM"))
        ystg = ctx.enter_context(tc.tile_pool(name="ystg", bufs=4))
        psum = ctx.enter_context(tc.tile_pool(name="psum", bufs=4, space="PSUM"))

        # scales^T: one DMA loads all of scales partition-split, then PE
        # transposes + ACT copies build scT [KO, OUT], bounced through DRAM
        # so per-oc row-blocks can be reloaded contiguously onto partition 0.
        ident = const.tile([P, P], FP16)
        make_identity(nc, ident)
        snat = const.tile([P, OUT // P, KO], FP16)
        sv = s[:, :].rearrange("(oo p) g -> p oo g", p=P)
        nc.gpsimd.dma_start(out=snat[:], in_=sv)
        scT = const.tile([KO, OUT], FP16)
        for o2 in range(OUT // P):
            pst = scps.tile([KO, P], FP16, tag="pst")
            nc.tensor.transpose(pst[:], snat[:, o2, :], ident[:])
            nc.scalar.copy(out=scT[:, o2 * P:(o2 + 1) * P], in_=pst[:])
        sT_dram = dramp.tile([KO, OUT], FP16)
        nc.gpsimd.dma_start(out=sT_dram[:], in_=scT[:])

        ones = const.tile([1, P], FP16)
        nc.gpsimd.memset(ones[:], 1.0)

        # x^T resident in two DMA transposes: xT[p, ko, m] = x[m, ko*128+p]
        xT = xTp.tile([P, KO, M], FP16)
        xc = KO // 2
        for i in range(2):
            nc.sync.dma_start_transpose(out=xT[:, i * xc:(i + 1) * xc, :],
                                        in_=x[:, i * xc * P:(i + 1) * xc * P])

        # Dequant per o-panel: the scales row for each k-chunk is broadcast
        # across partitions by a rank-1 PE matmul (ones^T x row) into PSUM,
        # then one DVE multiply applies it in place. (A DMA with a step-0
        # partition source AP measurably poisons the DMA pipeline on HW, so
        # the broadcast runs on the PE instead.) Emission is software-
        # pipelined one o-chunk ahead so the next panel's broadcasts+muls
        # run during this panel's matmul block instead of stalling the PE
        # at each o-chunk boundary.
        def emit_prefetch(oc):
            osl = slice(oc * OC, (oc + 1) * OC)
            wr3 = wraw.tile([P, KO, OC], FP16, tag="wraw", name="wr3")
            scp = scp0.tile([1, KO, OC], FP16, tag="scp", name="scp")
            nc.scalar.dma_start(out=scp[:], in_=sT_dram[:, osl])
            KH = KO // 2
            for g in range(2):
                kg = slice(g * KH, (g + 1) * KH)
                nc.sync.dma_start_transpose(
                    out=wr3[:, kg, :], in_=w[osl, g * KH * P:(g + 1) * KH * P])
                for ko in range(g * KH, (g + 1) * KH):
                    psb = psb_pool.tile([P, OC], FP32, tag="psb", name="psb")
                    nc.tensor.matmul(psb[:], ones[:], scp[0:1, ko, :],
                                     start=True, stop=True)
                    nc.vector.tensor_mul(wr3[:, ko, :], wr3[:, ko, :], psb[:])
            return wr3

        def emit_compute(oc, wr3):
            osl = slice(oc * OC, (oc + 1) * OC)
            for m in range(MT):
                pt = psum.tile([P, OC], FP32, name="pt")
                for ko in range(KO):
                    nc.tensor.matmul(
                        pt[:],
                        xT[:, ko, m * P:(m + 1) * P],
                        wr3[:, ko, :],
                        start=(ko == 0),
                        stop=(ko == KO - 1),
                    )
                yt = ystg.tile([P, OC], FP16, name="yt")
                nc.scalar.copy(out=yt[:], in_=pt[:])
                nc.scalar.dma_start(out=y[m * P:(m + 1) * P, osl], in_=yt[:])

        cur = emit_prefetch(0)
        for oc in range(NOC):
            nxt = emit_prefetch(oc + 1) if oc + 1 < NOC else None
            emit_compute(oc, cur)
            cur = nxt

    if split_waits:
        _split_multiwait_insts(nc)
    return nc


def _get_runner():
    """Compile once; return a reusable callable mapping per-core input maps
    to per-core output maps (modeled on bass2jax.run_bass_via_pjrt)."""
    global _RUNNER
    if _RUNNER is not None:
        return _RUNNER

    import jax
    from jax.experimental.shard_map import shard_map
    from jax.sharding import Mesh, PartitionSpec
    from concourse import bass2jax

    nc = _build()
    bass2jax.install_neuronx_cc_hook()

    partition_name = nc.partition_id_tensor.name if nc.partition_id_tensor else None
    in_names, out_names, out_avals, zero_shapes = [], [], [], []
    for alloc in nc.m.functions[0].allocations:
        if not isinstance(alloc, mybir.MemoryLocationSet):
            continue
        name = alloc.memorylocations[0].name
        if alloc.kind == "ExternalInput":
            if name != partition_name:
                in_names.append(name)
        elif alloc.kind == "ExternalOutput":
            shape = tuple(alloc.tensor_shape)
            dtype = mybir.dt.np(alloc.dtype)
            out_names.append(name)
            out_avals.append(jax.core.ShapedArray(shape, dtype))
            zero_shapes.append((shape, dtype))
    n_params = len(in_names)
    n_outs = len(out_names)
    all_names = in_names + out_names
    if partition_name is not None:
        all_names = all_names + [partition_name]
    donate = tuple(range(n_params, n_params + n_outs))

    def _make_body(reps):
        def _body(*args):
            ins = list(args[:n_params])
            outs = list(args[n_params:n_params + n_outs])
            for _ in range(reps):
                operands = ins + outs
                if partition_name is not None:
                    operands.append(bass2jax.partition_id_tensor())
                outs = list(bass2jax._bass_exec_p.bind(
                    *operands,
                    out_avals=tuple(out_avals),
                    in_names=tuple(all_names),
                    out_names=tuple(out_names),
                    lowering_input_output_aliases=(),
                    sim_require_finite=True,
                    sim_require_nnan=True,
                    nc=nc,
                ))
            return tuple(outs)
        return _body

    devices = jax.devices()[:NCORES]
    mesh = Mesh(np.asarray(devices), ("core",))

    def _make_exec(reps):
        return jax.jit(
            shard_map(
                _make_body(reps),
                mesh=mesh,
                in_specs=(PartitionSpec("core"),) * (n_params + n_outs),
                out_specs=(PartitionSpec("core"),) * n_outs,
                check_rep=False,
            ),
            donate_argnums=donate,
            keep_unused=True,
        )

    sharded = _make_exec(1)
    _exec_cache = {1: sharded}
    from jax.sharding import NamedSharding
    shard = NamedSharding(mesh, PartitionSpec("core"))

    class Runner:
        def __init__(self):
            self.in_names = in_names
            self.out_names = out_names

        def put_inputs(self, in_maps):
            """Concat per-core inputs and place them on the mesh."""
            import jax as _jax
            concat_in = [
                np.concatenate([np.asarray(m[name]) for m in in_maps], axis=0)
                for name in in_names
            ]
            return [_jax.device_put(a, shard) for a in concat_in]

        def fresh_outs(self):
            import jax as _jax
            return [
                _jax.device_put(np.zeros((NCORES * sh[0], *sh[1:]), dt), shard)
                for sh, dt in zero_shapes
            ]

        def exec_dev(self, dev_in, dev_outs, reps=1):
            """Device step(s). dev_outs is donated; returns new out arrays
            (same shape/sharding — reusable as the next call's dev_outs,
            since the kernel overwrites every output element). reps>1
            chains that many NEFF executions inside one dispatch."""
            if reps not in _exec_cache:
                _exec_cache[reps] = _make_exec(reps)
            return _exec_cache[reps](*dev_in, *dev_outs)

        def run(self, in_maps):
            dev_in = self.put_inputs(in_maps)
            out_arrs = self.exec_dev(dev_in, self.fresh_outs())
            return [
                {
                    name: np.asarray(out_arrs[i]).reshape(
                        NCORES, *out_avals[i].shape)[c]
                    for i, name in enumerate(out_names)
                }
                for c in range(NCORES)
            ]

    _RUNNER = Runner()
    return _RUNNER


def kernel(x, weight, scales):
    runner = _get_runner()
    xf = np.ascontiguousarray(np.asarray(x, dtype=np.float16).reshape(B * S, IN))
    w = np.ascontiguousarray(np.asarray(weight, dtype=np.float16))
    s = np.ascontiguousarray(np.asarray(scales, dtype=np.float16))
    in_maps = [
        {"x": xf[c * M:(c + 1) * M], "w": w, "s": s} for c in range(NCORES)
    ]
    outs = runner.run(in_maps)
    yf = np.concatenate([outs[c]["y"] for c in range(NCORES)], axis=0)
    return yf.reshape(B, S, OUT).astype(np.float16)

